# revision 8
# baseline (speedup 1.0000x reference)
"""Trainium2 Bass kernel for nn_Critic_Mix (dense MLP critic with teacher mixing).

v2 strategy (see kernel_v1_backup.py for the baseline):
  - Pure data parallel: batch (B=262144) sharded across 8 cores (32768 rows each).
  - Rows processed in PAIRS of 512-row tiles; every PSUM evacuation covers
    [128,1024] (two banks) to amortize the ~300-400ns fixed per-instruction
    cost of the ACT/DVE PSUM-drain path (PSUM-source ops run at 1 elem/cyc).
  - mix_factor m ~= 0.037 damps every teacher contribution by c_k = m*P_k,
    so the whole teacher L2 stage runs in fp8(e4m3):
      * standalone teacher z2k matmuls: fp8 weights x fp8 rh1 activations.
      * the main-path  sum(c_k * z2k)  uses fp8 DoubleRow pairs - two K=128
        matmuls fused into one K=256 pass (2x PE throughput) - accumulated
        on top of an fp16 h1 @ W2 matmul in the same PSUM bank.
  - All scale factors are powers of two chosen at runtime from a host-side
    subsample forward pass; they fold exactly into neighboring weights.
  - L1 mixsum folded into one effective main weight (linear, exact).
  - L3: all 5 matmuls per head accumulate into one PSUM bank; the two heads'
    64-wide matmuls run concurrently in disjoint PE column groups.
  - L4 ([64]->[1]) via 16 one-hot-column weight variants accumulating 8
    row-tiles x 2 heads into one PSUM bank; L4 matmuls for a pair are
    deferred to the next pair's L3 window to bridge the single-buffer ps3
    drain latency.
"""

import os
import sys
from contextlib import ExitStack

import numpy as np

for _p in ("/opt/trn_rl_repo",):
    if _p not in sys.path and os.path.isdir(_p):
        sys.path.insert(0, _p)

import ml_dtypes

import concourse.bass as bass
import concourse.tile as tile
from concourse import bacc, mybir
from concourse._compat import with_exitstack
from concourse.bass import ts
from concourse.bass_utils import run_bass_kernel_spmd

# Problem constants (hardcoded; kernel.py must be self-contained).
B = 262144
NCORES = 8
CB = B // NCORES          # rows per core
NT = 512                  # moving-dim tile (one PSUM bank of fp32)
DIN = 128                 # xu feature dim (96 + 32)
H1 = 128
H2 = 64
K = 4

F32 = mybir.dt.float32
F16 = mybir.dt.float16
F8 = mybir.dt.float8e4
E4M3 = ml_dtypes.float8_e4m3fn
AF = mybir.ActivationFunctionType
ALU = mybir.AluOpType
DR = mybir.MatmulPerfMode.DoubleRow


# ---------------------------------------------------------------------------
# Weight / bias column layouts (shared by host packing and kernel body)
# ---------------------------------------------------------------------------
def _wlayout():
    """fp16 weight columns."""
    off = {}
    cur = 0
    for h in (0, 1):
        for k in range(K):
            off[f"l1t{h}{k}"] = (cur, 128)
            cur += 128
        off[f"l1m{h}"] = (cur, 128)
        cur += 128
        off[f"l2h1m{h}"] = (cur, 128)
        cur += 128
        for k in range(K):
            off[f"l3t{h}{k}"] = (cur, 64)
            cur += 64
        off[f"l3m{h}"] = (cur, 64)
        cur += 64
    off["l4"] = (cur, 8 * 16)  # 8 variants x 16 cols (block-diag across heads)
    cur += 8 * 16
    return off, cur


# fp8 weights: 16 chunks of 128 cols. Chunks 0..7: standalone L2 teacher
# (h*4+k). Chunks 8..15: DoubleRow pairs - pair (h,p) occupies chunks
# 8+2*(h*2+p) and 8+2*(h*2+p)+1 (members k=2p and k=2p+1).
W8CH = 16


def _w8chunk_std(h, k):
    return h * 4 + k


def _w8chunk_dr(h, p):
    return 8 + 2 * (h * 2 + p)


def _blayout():
    off = {}
    cur = 0
    for h in (0, 1):
        for k in range(K):
            off[f"b1t{h}{k}"] = cur
            cur += 1
        off[f"b1m{h}"] = cur
        cur += 1
        for k in range(K):
            off[f"b2t{h}{k}"] = cur
            cur += 1
        off[f"b2m{h}"] = cur
        cur += 1
    off["b3cat"] = cur  # [b3 head0 (rows 0:64) | b3 head1 (rows 64:128)]
    cur += 1
    return off, cur


WOFF, WCOLS = _wlayout()
BOFF, BCOLS = _blayout()


def _pow2floor(x):
    return np.float32(2.0 ** np.clip(np.floor(np.log2(max(float(x), 1e-30))), -24, 24))


# ---------------------------------------------------------------------------
# Host-side parameter folding
# ---------------------------------------------------------------------------
def prepare_params(inputs):
    """Pack folded weights/biases.

    Returns (wts [128,WCOLS] fp16, w8 [128,W8CH*128] e4m3, biasv [128,BCOLS]
    fp32, (b4, b8)).
    """
    m = np.float32(np.asarray(inputs["mix_factor"]).reshape(-1)[0])
    P = np.asarray(inputs["teacher_P"], np.float32).reshape(K)
    om = np.float32(1.0) - m
    c = (m * P).astype(np.float32)  # [K], >= 0

    wts = np.zeros((128, WCOLS), np.float32)
    w8 = np.zeros((128, W8CH * 128), np.float32)
    biasv = np.zeros((128, BCOLS), np.float32)

    def wput(name, arr):
        o, w = WOFF[name]
        arr = np.asarray(arr, np.float32)
        assert arr.shape[1] == w, (name, arr.shape, w)
        wts[: arr.shape[0], o : o + w] = arr

    def bput(name, vec, base=0):
        vec = np.asarray(vec, np.float32).reshape(-1)
        biasv[base : base + vec.shape[0], BOFF[name]] = vec

    # subsample of rows for activation-range estimation (deterministic)
    x = np.asarray(inputs["x"], np.float32)
    u = np.asarray(inputs["u"], np.float32)
    n_s = 4096
    step = max(1, B // n_s)
    xu_s = np.concatenate([x[::step], u[::step]], axis=1)[:n_s]  # [n_s,128]

    heads = [
        ("W1", "b1", "W2", "b2", "W3", "b3", "W4", "b4", "tW1", "tb1", "tW2", "tb2", "tW3", "tb3"),
        ("W5", "b5", "W6", "b6", "W7", "b7", "W8", "b8", "tW5", "tb5", "tW6", "tb6", "tW7", "tb7"),
    ]
    out_biases = []
    l4 = np.zeros((128, 8 * 16), np.float32)
    for h, names in enumerate(heads):
        (Wa, ba, Wb, bb, Wc, bc, Wd, bd, tWa, tba, tWb, tbb, tWc, tbc) = (
            np.asarray(inputs[n], np.float32) for n in names
        )
        # --- subsample forward for ranges
        rh1_s = [np.maximum(xu_s @ tWa[k].T + tba[k], 0.0) for k in range(K)]
        rh2_s = [np.maximum(rh1_s[k] @ tWb[k].T + tbb[k], 0.0) for k in range(K)]
        h1_s = np.maximum(
            xu_s @ (om * Wa + m * np.einsum("k,koi->oi", P, tWa)).T
            + (om * ba + m * (P[:, None] * tba).sum(0)),
            0.0,
        )
        z2m_s = h1_s @ (om * Wb).T + sum(
            c[k] * (rh1_s[k] @ tWb[k].T) for k in range(K)
        ) + (om * bb + (c[:, None] * tbb).sum(0))
        Mh2 = float(np.maximum(z2m_s, 0.0).max()) + 1e-6

        s1 = np.zeros(K, np.float32)
        alpha = np.zeros(K, np.float32)
        for k in range(K):
            M1k = float(rh1_s[k].max()) + 1e-6
            s1[k] = _pow2floor(64.0 / M1k)
            M2k = float(rh2_s[k].max()) + 1e-6
            maxW3k = float(np.abs(tWc[k]).max()) + 1e-30
            a_cap_rng = 8192.0 / max(float(c[k]) * M2k * 1.5, 1e-30)
            a_cap_w3 = maxW3k / 1.2e-4
            alpha[k] = _pow2floor(min(a_cap_rng, a_cap_w3, 1e7))
        beta = _pow2floor(
            min(om * float(np.abs(Wc).max()) / 1.2e-4, 8192.0 / (Mh2 * 1.5))
        )

        # --- L1: teachers scaled by s1k; main folded with the layer-1 mixsum.
        for k in range(K):
            wput(f"l1t{h}{k}", s1[k] * tWa[k].T)
            bput(f"b1t{h}{k}", s1[k] * tba[k])
        W1eff = om * Wa + m * np.einsum("k,koi->oi", P, tWa)
        b1eff = om * ba + m * (P[:, None] * tba).sum(0)
        wput(f"l1m{h}", W1eff.T)
        bput(f"b1m{h}", b1eff)

        # --- L2 standalone teachers (fp8):  psum = alpha_k*c_k*(z2k - b2k)
        for k in range(K):
            wq = (alpha[k] * c[k] / s1[k]) * tWb[k].T
            assert float(np.abs(wq).max()) < 200.0, ("w8std overflow", h, k)
            w8[:, _w8chunk_std(h, k) * 128 : (_w8chunk_std(h, k) + 1) * 128] = wq
            bput(f"b2t{h}{k}", alpha[k] * c[k] * tbb[k])

        # --- L2 main chain: fp16 h1-term + 2 fp8 DoubleRow pairs
        wput(f"l2h1m{h}", (beta * om * Wb).T)
        for p in range(2):
            ch = _w8chunk_dr(h, p)
            for j in range(2):
                k = 2 * p + j
                wq = (beta * c[k] / s1[k]) * tWb[k].T
                assert float(np.abs(wq).max()) < 200.0, ("w8dr overflow", h, p, j)
                w8[:, (ch + j) * 128 : (ch + j + 1) * 128] = wq
        bput(f"b2m{h}", beta * (om * bb + (c[:, None] * tbb).sum(0)))

        # --- L3: teachers absorb 1/alpha_k (c_k cancels m*P_k exactly);
        # main absorbs 1/beta.
        for k in range(K):
            wput(f"l3t{h}{k}", (tWc[k] / alpha[k]).T)
        wput(f"l3m{h}", ((om / beta) * Wc).T)
        bput("b3cat", om * bc + m * (P[:, None] * tbc).sum(0), base=64 * h)

        # --- L4 variants: variant j (tile t%8) has w4(head0) in rows 0:64 of
        # col j and w8(head1) in rows 64:128 of col 8+j.
        for j in range(8):
            l4[64 * h : 64 * h + 64, j * 16 + 8 * h + j] = Wd[0]
        out_biases.append(np.float32(bd[0]))

    o, w = WOFF["l4"]
    wts[:, o : o + w] = l4
    w8q = np.clip(w8, -240.0, 240.0).astype(E4M3)
    return wts.astype(np.float16), w8q, biasv, out_biases


def prepare_xut(inputs):
    x = np.asarray(inputs["x"], np.float32)
    u = np.asarray(inputs["u"], np.float32)
    xu = np.concatenate([x, u], axis=1)  # [B, 128]
    return np.ascontiguousarray(xu.T).astype(np.float16)  # [128, B]


# ---------------------------------------------------------------------------
# Kernel body
# ---------------------------------------------------------------------------
@with_exitstack
def _critic_body(ctx: ExitStack, tc, out_ap, xu_ap, wts_ap, w8_ap, bias_ap, tiles: int):
    nc = tc.nc
    pairs = tiles // 2

    const = ctx.enter_context(tc.tile_pool(name="const", bufs=1))
    xup = ctx.enter_context(tc.tile_pool(name="xup", bufs=3))
    rhp = ctx.enter_context(tc.tile_pool(name="rhp", bufs=2))
    actp = ctx.enter_context(tc.tile_pool(name="actp", bufs=2))
    h3p = ctx.enter_context(tc.tile_pool(name="h3p", bufs=4))
    psp = ctx.enter_context(tc.tile_pool(name="psp", bufs=3, space=bass.MemorySpace.PSUM))
    ps3p = ctx.enter_context(tc.tile_pool(name="ps3p", bufs=1, space=bass.MemorySpace.PSUM))
    ps4p = ctx.enter_context(tc.tile_pool(name="ps4p", bufs=1, space=bass.MemorySpace.PSUM))

    wts = const.tile([128, WCOLS], F16)
    nc.gpsimd.dma_start(wts[:], wts_ap[:])
    w8t = const.tile([128, W8CH, 128], F8)
    nc.gpsimd.dma_start(w8t[:], w8_ap[:])
    biasv = const.tile([128, BCOLS], F32)
    nc.gpsimd.dma_start(biasv[:], bias_ap[:])

    def w(name):
        o, wd = WOFF[name]
        return wts[:, o : o + wd]

    def bvec(name, parts=128):
        col = BOFF[name]
        return biasv[0:parts, col : col + 1]

    def drain(dst, src, bname, eng, parts=128):
        # dst = relu(src + bias)
        if eng == "act":
            nc.scalar.activation(dst, src, AF.Relu, bias=bvec(bname, parts), scale=1.0)
        else:
            nc.vector.tensor_scalar(
                out=dst, in0=src, scalar1=bvec(bname, parts), scalar2=0.0,
                op0=ALU.add, op1=ALU.max,
            )

    l4o, _ = WOFF["l4"]
    pending_l4 = []  # (tile_idx, h3_tile), emitted a few chunks after the drain
    ps4 = None

    def emit_l4():
        nonlocal ps4
        for t, h3t in pending_l4:
            j = t % 8
            if j == 0:
                ps4 = ps4p.tile([16, NT], F32, tag="ps4")
            nc.tensor.matmul(
                ps4[:], wts[:, l4o + j * 16 : l4o + (j + 1) * 16], h3t[:],
                start=(j == 0), stop=(j == 7),
            )
            if j == 7:
                osb = actp.tile([16, NT], F32, tag="osb")
                nc.vector.tensor_scalar(
                    out=osb[:], in0=ps4[:], scalar1=0.0, scalar2=None, op0=ALU.add
                )
                nc.gpsimd.dma_start(out_ap[:, ts(t // 8, NT)], osb[:])
        pending_l4.clear()

    def l12_steps(p, state):
        """Pair p's L1+L2 blocks; yields after each PSUM-slot chunk."""
        xu = xup.tile([128, 2, NT], F16, tag="xu")
        nc.gpsimd.dma_start(xu[:], xu_ap[:, ts(p, 2 * NT)])

        rh1s, h1s = [], []
        for h in (0, 1):
            # ---- L1: 4 teachers (scaled, fp8 out) + folded main (fp16 out)
            rh1 = rhp.tile([128, K, 2, NT], F8, tag=f"rh1_{h}")
            for k in range(K):
                ps = psp.tile([128, 2, NT], F32, tag="ps")
                nc.tensor.matmul(ps[:, 0, :], w(f"l1t{h}{k}"), xu[:, 0, :], start=True, stop=True)
                nc.tensor.matmul(ps[:, 1, :], w(f"l1t{h}{k}"), xu[:, 1, :], start=True, stop=True)
                eng = ("act", "dve")[(k + h) % 2]
                drain(rh1[:, k, :, :], ps[:], f"b1t{h}{k}", eng)
                yield True
            psm = psp.tile([128, 2, NT], F32, tag="ps")
            nc.tensor.matmul(psm[:, 0, :], w(f"l1m{h}"), xu[:, 0, :], start=True, stop=True)
            nc.tensor.matmul(psm[:, 1, :], w(f"l1m{h}"), xu[:, 1, :], start=True, stop=True)
            h1 = actp.tile([128, 2, NT], F16, tag=f"h1_{h}")
            drain(h1[:], psm[:], f"b1m{h}", ("act", "dve")[h])
            rh1s.append(rh1)
            h1s.append(h1)
            yield True

        for h in (0, 1):
            rh1, h1 = rh1s[h], h1s[h]
            # ---- L2 standalone teachers (fp8 x fp8)
            rh2 = []
            for k in range(K):
                ch = _w8chunk_std(h, k)
                ps2 = psp.tile([128, 2, NT], F32, tag="ps")
                nc.tensor.matmul(ps2[:, 0, :], w8t[:, ch, :], rh1[:, k, 0, :], start=True, stop=True)
                nc.tensor.matmul(ps2[:, 1, :], w8t[:, ch, :], rh1[:, k, 1, :], start=True, stop=True)
                r2 = actp.tile([128, 2, NT], F16, tag=f"rh2_{h}_{k}")
                drain(r2[:], ps2[:], f"b2t{h}{k}", ("dve", "act")[(k + h) % 2])
                rh2.append(r2)
                yield True
            # ---- L2 main: fp16 h1-term + 2 fp8 DoubleRow pairs per tile
            ps2m = psp.tile([128, 2, NT], F32, tag="ps")
            for q in (0, 1):
                nc.tensor.matmul(ps2m[:, q, :], w(f"l2h1m{h}"), h1[:, q, :], start=True, stop=False)
                for pp in (0, 1):
                    ch = _w8chunk_dr(h, pp)
                    nc.tensor.matmul(
                        ps2m[:, q, :], w8t[:, ch : ch + 2, :],
                        rh1[:, 2 * pp : 2 * pp + 2, q, :],
                        start=False, stop=(pp == 1), perf_mode=DR,
                    )
            h2 = actp.tile([128, 2, NT], F16, tag=f"h2_{h}")
            drain(h2[:], ps2m[:], f"b2m{h}", "act")
            state[f"rh2_{h}"] = rh2
            state[f"h2_{h}"] = h2
            yield False  # keep DoubleRow LDWEIGHTS chains unbroken

    def l34_steps(p, state):
        """Pair p's L3+L4 blocks (uses state captured by l12_steps(p));
        yields after every ~1 PE stream."""
        rh2s = [state["rh2_0"], state["rh2_1"]]
        h2s = [state["h2_0"], state["h2_1"]]
        # L4s of the previous pair: their h3 drains completed long ago.
        emit_l4()
        yield
        for q in (0, 1):
            ps3 = ps3p.tile([128, NT], F32, tag="ps3")
            for k in range(K):
                for h in (0, 1):
                    nc.tensor.matmul(ps3[64 * h : 64 * h + 64, :], w(f"l3t{h}{k}"),
                                     rh2s[h][k][:, q, :], start=(k == 0), stop=False,
                                     tile_position=(0, 64 * h), skip_group_check=True)
                yield
            for h in (0, 1):
                nc.tensor.matmul(ps3[64 * h : 64 * h + 64, :], w(f"l3m{h}"),
                                 h2s[h][:, q, :], start=False, stop=True,
                                 tile_position=(0, 64 * h), skip_group_check=True)
            h3 = h3p.tile([128, NT], F16, tag="h3")
            drain(h3[:], ps3[:], "b3cat", ("act", "dve")[q])
            pending_l4.append((2 * p + q, h3))
            yield

    # Software pipeline: interleave pair p's L1/L2 with pair (p-1)'s L3/L4 so
    # the PE always has independent work while PSUM-slot drains are in flight.
    states = [dict() for _ in range(pairs)]
    gb = None
    RATIO = 11.0 / 18.0  # l34 yields per interleavable l12 yield
    for p in range(pairs):
        ga = l12_steps(p, states[p])
        acc = 0.0
        for ok in ga:
            if gb is not None and ok:
                acc += RATIO
                while acc >= 1.0:
                    next(gb, None)
                    acc -= 1.0
        gb = l34_steps(p, states[p])
    if gb is not None:
        for _ in gb:
            pass
    emit_l4()


def build_nc(cb=CB):
    """Build + compile the per-core program for cb rows (cb % (8*NT) == 0)."""
    assert cb % (8 * NT) == 0
    tiles = cb // NT
    nc = bacc.Bacc(
        "TRN2",
        target_bir_lowering=False,
        debug=False,
        enable_asserts=False,
        num_devices=NCORES,
    )
    xu_ap = nc.dram_tensor("xut", [128, cb], F16, kind="ExternalInput").ap()
    wts_ap = nc.dram_tensor("wts", [128, WCOLS], F16, kind="ExternalInput").ap()
    w8_ap = nc.dram_tensor("w8", [128, W8CH * 128], F8, kind="ExternalInput").ap()
    bias_ap = nc.dram_tensor("biasv", [128, BCOLS], F32, kind="ExternalInput").ap()
    out_ap = nc.dram_tensor("out", [16, cb // 8], F32, kind="ExternalOutput").ap()
    with tile.TileContext(nc) as tc:
        _critic_body(tc, out_ap, xu_ap, wts_ap, w8_ap, bias_ap, tiles)
    nc.compile()
    return nc


def unscramble_out(out_c):
    """[16, cb//8] device layout -> (y1 [cb], y2 [cb])."""
    g = out_c.shape[1] // NT
    ys = []
    for h in (0, 1):
        a = out_c[8 * h : 8 * h + 8].reshape(8, g, NT)
        ys.append(np.ascontiguousarray(a.transpose(1, 0, 2)).reshape(-1))
    return ys


_NC_CACHE = {}
LAST_RESULTS = None  # BassKernelResults of the most recent run (for profiling)


def kernel(**inputs):
    global LAST_RESULTS
    wts, w8, biasv, (b4, b8) = prepare_params(inputs)
    xut = prepare_xut(inputs)

    if CB not in _NC_CACHE:
        _NC_CACHE[CB] = build_nc(CB)
    nc = _NC_CACHE[CB]

    in_maps = [
        {
            "xut": np.ascontiguousarray(xut[:, c * CB : (c + 1) * CB]),
            "wts": wts,
            "w8": w8,
            "biasv": biasv,
        }
        for c in range(NCORES)
    ]
    res = run_bass_kernel_spmd(
        nc,
        in_maps,
        list(range(NCORES)),
        trace=bool(os.environ.get("BASS_TRACE")),
    )
    LAST_RESULTS = res

    y1 = np.empty(B, np.float32)
    y2 = np.empty(B, np.float32)
    for c in range(NCORES):
        a, b = unscramble_out(res.results[c]["out"])
        y1[c * CB : (c + 1) * CB] = a
        y2[c * CB : (c + 1) * CB] = b
    y1 += b4
    y2 += b8
    return (y1[:, None], y2[:, None])
